# revision 34
# baseline (speedup 1.0000x reference)
"""Chamfer loss (nn_ChamferLoss_45157286150461) Trainium2 Bass kernel.

Math (matches the reference):
    P[b,i,j] = ||gts[b,i]||^2 + ||preds[b,j]||^2 - 2 gts[b,i].preds[b,j]
    out = mean_j min_i P  +  mean_i min_j P       (means over all b,j / b,i)

Sharding: data-parallel over batch. 8 cores x 2 batches each. Each core
returns one f32 partial = sum(min_i P) + sum(min_j P) over its two
batches; the host sums the 8 partials and divides by B*N.

Device-side per batch:
  - PE: fp16 hi/lo-split augmented matmul (K=13):
        u = [xs_h xs_h xs_l sx_h sx_l 1 1], v = [y_h y_l y_h 1 1 sy_h sy_l]
    with xs = -2x, so u.v = -2x.y + |x|^2 + |y|^2 up to ~1e-6 (the
    dropped xs_l*y_l term). 1 col/cycle vs fp32's 1/4 rate.
  - ScalarE: converts [128,JG] PSUM tiles to fp16 SBUF (the only engine
    that can drain PSUM without stealing VectorE throughput).
  - VectorE (all fp16 SBUF, fd=512 ops -- the measured 2x-mode sweet
    spot): tensor_tensor min-accumulate into M[128,4096] (min-over-i
    direction) and into per-i-tile R[128,512] (min-over-j direction),
    plus one tensor_reduce per i-tile for dr.
  - Epilogue: PE-transpose of M chunks + free-dim min for the partition
    direction; sums via reduce-add and a ones-matmul partition sum.

HW-measured notes (axon-tunneled trn2, For_i-slope timing): fp32 matmul
is 1/4 rate (hence the fp16 split); tensor_scalar/tensor_tensor_reduce
accum_out are ~10x slower than modeled (avoided); gpsimd elementwise
does not compile in this toolchain; DVE TT fp16 fd=512 = ~253ns.
"""

import os
import sys
from contextlib import ExitStack

for _p in ("/opt/trn_rl_repo", "/root/.axon_site/_ro/trn_rl_repo"):
    if os.path.isdir(_p) and _p not in sys.path:
        sys.path.insert(0, _p)

import numpy as np

import concourse.bass as bass  # noqa: F401
import concourse.tile as tile
from concourse import bacc, mybir
from concourse.bass_utils import run_bass_kernel_spmd

f32 = mybir.dt.float32
f16 = mybir.dt.float16
AX = mybir.AxisListType
OP = mybir.AluOpType
ACTF = mybir.ActivationFunctionType

N_CORES = 8
B = 16
N = 4096
D = 3
BPC = B // N_CORES  # batches per core
NT = 2 * BPC        # stacked tensor count (x b0, x b1, y b0, y b1)
P = 128             # i-tile (PSUM partition dim)
JW = 512            # j-tile per matmul
JG = int(os.environ.get("CHAMFER_JG", "512"))  # j-group per PSUM tile
NIT = N // P        # 32
NJG = N // JG
BIG = 60000.0       # > any squared distance here, < fp16 max
KC = 13             # augmented contraction rows



def build_program(do_compile=True, loop_reps=None, unroll_reps=1):
    nc = bacc.Bacc("TRN2", target_bir_lowering=False, debug=False)

    # Stacked inputs: xq rows = [x0(3) x1(3) y0(3) y1(3)] transposed comps,
    # wq = per-tensor [32, 384] point-major blocks stacked on partitions.
    xq_d = nc.dram_tensor("xq", [NT * D, N], f32, kind="ExternalInput")
    wq_d = nc.dram_tensor("wq", [NT * NIT, D * P], f32, kind="ExternalInput")
    ones_d = nc.dram_tensor("ones16", [2, N], f16, kind="ExternalInput")
    ident_d = nc.dram_tensor("ident", [P, P], f16, kind="ExternalInput")
    out_d = nc.dram_tensor("out", [1, 1], f32, kind="ExternalOutput")

    with ExitStack() as ctx:
        tc = ctx.enter_context(tile.TileContext(nc))
        consts = ctx.enter_context(tc.tile_pool(name="consts", bufs=1))
        prep = ctx.enter_context(tc.tile_pool(name="prep", bufs=1))
        uvp = ctx.enter_context(tc.tile_pool(name="uv", bufs=1))
        tpool = ctx.enter_context(
            tc.tile_pool(name="tconv", bufs=int(os.environ.get("CHAMFER_TBUFS", "4")))
        )
        mpool = ctx.enter_context(tc.tile_pool(name="mmin", bufs=2))
        accp = ctx.enter_context(tc.tile_pool(name="acc", bufs=2))
        resp = ctx.enter_context(tc.tile_pool(name="res", bufs=1))
        ps_mm = ctx.enter_context(
            tc.tile_pool(
                name="psmm",
                bufs=int(os.environ.get("CHAMFER_PSMM_BUFS", "5")),
                space="PSUM",
            )
        )
        trmode = os.environ.get("CHAMFER_TRMODE", "pe")
        ps_tr = None
        if trmode == "pe":
            ps_tr = ctx.enter_context(
                tc.tile_pool(
                    name="pstr",
                    bufs=int(os.environ.get("CHAMFER_PSTR_BUFS", "2")),
                    space="PSUM",
                )
            )
        trp = ctx.enter_context(tc.tile_pool(name="trsb", bufs=2))
        ps_fin = ctx.enter_context(tc.tile_pool(name="psfin", bufs=1, space="PSUM"))

        ident_sb = consts.tile([P, P], f16)
        nc.sync.dma_start(ident_sb[:], ident_d[:])
        ones_sb = consts.tile([2, N], f16)
        nc.sync.dma_start(ones_sb[:], ones_d[:])
        ones_col = consts.tile([P, 1], f32)
        nc.vector.memset(ones_col[:], 1.0)
        res = resp.tile([1, BPC], f32)
        if os.environ.get("CHAMFER_ABLATE", "") == "preponly":
            nc.vector.memset(res[:], 0.0)

        if loop_reps is not None:
            ctx.enter_context(tc.For_i(0, loop_reps, 1))

        # ---------- prep: hi/lo split of -2x / y and the squared norms ----
        # DMAs rotate across the two HWDGE queues (SP / ACT engine)
        _qs = [nc.sync, nc.scalar]
        _qi = [0]

        def dma(out_ap, in_ap):
            _qs[_qi[0] % len(_qs)].dma_start(out_ap, in_ap)
            _qi[0] += 1

        wq = prep.tile([NT * NIT, D * P], f32, tag="wq")
        nc.sync.dma_start(wq[:], wq_d[:])
        xq = prep.tile([NT * D, N], f32, tag="xq")
        nc.scalar.dma_start(xq[:], xq_d[:])

        # squared norms (longest chain first)
        sq = prep.tile([NT * NIT, D * P], f32, tag="sq")
        nc.vector.tensor_mul(sq[:], wq[:], wq[:])
        sw = prep.tile([NT * NIT, P], f32, tag="sw")
        nc.vector.tensor_reduce(
            sw[:], sq[:].rearrange("p (k c) -> p k c", c=D), axis=AX.X, op=OP.add
        )
        ssf = prep.tile([NT, N], f32, tag="ssf")
        nc.sync.dma_start(ssf[:], sw[:])  # one flatten for all 4 tensors
        ssh = prep.tile([NT, N], f16, tag="ssh")
        nc.scalar.activation(ssh[:], ssf[:], ACTF.Copy)
        ssl = prep.tile([NT, N], f16, tag="ssl")
        nc.vector.tensor_sub(ssl[:], ssf[:], ssh[:])

        # scale the x rows (tensors 0..BPC-1) by -2
        nc.vector.tensor_scalar_mul(
            xq[0 : BPC * D, :], xq[0 : BPC * D, :], -2.0
        )
        hi = prep.tile([NT * D, N], f16, tag="hi")
        nc.scalar.activation(hi[:], xq[:], ACTF.Copy)
        lo = prep.tile([NT * D, N], f16, tag="lo")
        nc.vector.tensor_sub(lo[:], xq[:], hi[:])

        # assemble u (x side) / v (y side) tiles per batch
        uv = []
        for b in range(BPC):
            xi, yi = b, BPC + b  # tensor indices in the stacks
            u = uvp.tile([KC, N], f16, tag=f"u{b}")
            dma(u[0:3, :], hi[xi * D : xi * D + D, :])
            dma(u[3:6, :], hi[xi * D : xi * D + D, :])
            dma(u[6:9, :], lo[xi * D : xi * D + D, :])
            dma(u[9:10, :], ssh[xi : xi + 1, :])
            dma(u[10:11, :], ssl[xi : xi + 1, :])
            dma(u[11:13, :], ones_sb[:])
            v = uvp.tile([KC, N], f16, tag=f"v{b}")
            dma(v[0:3, :], hi[yi * D : yi * D + D, :])
            dma(v[3:6, :], lo[yi * D : yi * D + D, :])
            dma(v[6:9, :], hi[yi * D : yi * D + D, :])
            dma(v[9:11, :], ones_sb[:])
            dma(v[11:12, :], ssh[yi : yi + 1, :])
            dma(v[12:13, :], ssl[yi : yi + 1, :])
            uv.append((u, v))

        ablate = os.environ.get("CHAMFER_ABLATE", "")
        skip_act = ablate == "mmonly"
        skip_tt = ablate in ("nodl", "noredux", "mmonly")
        skip_ts = ablate in ("nodr", "noredux", "mmonly")
        skip_main = ablate == "preponly"

        # ---------- main ----------
        for b in [] if skip_main else [
            bb for _ in range(unroll_reps) for bb in range(BPC)
        ]:
            u, v = uv[b]
            M = mpool.tile([P, N], f16, tag="M")
            DR = accp.tile([P, NIT], f32, tag="DR")
            DL = accp.tile([P, NIT], f32, tag="DL")
            if skip_act:
                nc.vector.memset(M[:], BIG)
            if skip_ts:
                nc.vector.memset(DR[:], BIG)
            for it in range(NIT):
                lhsT = u[:, it * P : (it + 1) * P]
                Rt = accp.tile([P, JW], f16, tag="R")
                R = Rt[:]
                for jg in range(NJG):
                    ps = ps_mm.tile([P, JG], f32, tag="ps")
                    for h in range(JG // JW):
                        j0 = jg * JG + h * JW
                        nc.tensor.matmul(
                            ps[:, h * JW : (h + 1) * JW],
                            lhsT,
                            v[:, j0 : j0 + JW],
                            start=True,
                            stop=True,
                        )
                    msl = M[:, jg * JG : (jg + 1) * JG]
                    if skip_act:
                        continue
                    rinit = os.environ.get("CHAMFER_RINIT", "1") == "1"
                    nh = JG // JW
                    if it == 0:
                        nc.scalar.activation(msl, ps[:], ACTF.Copy)
                        srcs = [msl[:, h * JW : (h + 1) * JW] for h in range(nh)]
                    elif rinit and jg == 0 and not skip_ts:
                        # convert the first j-chunk directly into R:
                        # serves as the dr-accumulator init for free
                        srcs = []
                        for h in range(nh):
                            if h == 0:
                                nc.scalar.activation(R, ps[:, 0:JW], ACTF.Copy)
                                srcs.append(R)
                            else:
                                th = tpool.tile([P, JW], f16, tag="T")
                                nc.scalar.activation(
                                    th[:], ps[:, h * JW : (h + 1) * JW], ACTF.Copy
                                )
                                srcs.append(th[:])
                    else:
                        t = tpool.tile([P, JG], f16, tag="T")
                        nc.scalar.activation(t[:], ps[:], ACTF.Copy)
                        srcs = [t[:, h * JW : (h + 1) * JW] for h in range(nh)]
                    dr_first = os.environ.get("CHAMFER_DR_FIRST", "0") == "1"
                    for h in range(JG // JW):
                        tch = srcs[h]

                        def emit_dl():
                            if it > 0 and not skip_tt:
                                nc.vector.tensor_tensor(
                                    msl[:, h * JW : (h + 1) * JW],
                                    tch,
                                    msl[:, h * JW : (h + 1) * JW],
                                    op=OP.min,
                                )

                        def emit_dr():
                            if skip_ts:
                                return
                            if jg == 0 and h == 0:
                                if tch is R:
                                    return  # R already holds this chunk
                                nc.vector.tensor_copy(R, tch)
                            else:
                                nc.vector.tensor_tensor(R, tch, R, op=OP.min)

                        if dr_first:
                            emit_dr()
                            emit_dl()
                        else:
                            emit_dl()
                            emit_dr()
                if not skip_ts:
                    nc.vector.tensor_reduce(
                        DR[:, it : it + 1], R, axis=AX.X, op=OP.min
                    )

            # ---- min over partitions (dl): transpose + free-dim min ----
            for k in range(NIT):
                if trmode == "pe":
                    pst = ps_tr.tile([P, P], f16, tag="pst")
                    nc.tensor.transpose(
                        pst[:], M[:, k * P : (k + 1) * P], ident_sb[:]
                    )
                    nc.vector.tensor_reduce(
                        DL[:, k : k + 1], pst[:], axis=AX.X, op=OP.min
                    )
                else:
                    tst = trp.tile([P, P], f16, tag="tst")
                    _qs[(_qi[0] + k) % len(_qs)].dma_start(
                        tst[:], M[:, k * P : (k + 1) * P], transpose=True
                    )
                    nc.vector.tensor_reduce(
                        DL[:, k : k + 1], tst[:], axis=AX.X, op=OP.min
                    )

            # ---- sums ----
            sm = accp.tile([P, 2], f32, tag="sm")
            nc.vector.tensor_reduce(sm[:, 0:1], DR[:], axis=AX.X, op=OP.add)
            nc.vector.tensor_reduce(sm[:, 1:2], DL[:], axis=AX.X, op=OP.add)
            sv = accp.tile([P, 1], f32, tag="sv")
            nc.vector.tensor_reduce(sv[:], sm[:], axis=AX.X, op=OP.add)
            psf = ps_fin.tile([1, 1], f32, tag="psf")
            nc.tensor.matmul(psf[:], sv[:], ones_col[:], start=True, stop=True)
            nc.scalar.activation(res[:, b : b + 1], psf[:], ACTF.Copy)

        outsb = resp.tile([1, 1], f32)
        nc.vector.tensor_reduce(outsb[:], res[:], axis=AX.X, op=OP.add)
        nc.sync.dma_start(out_d[:], outsb[:])

    if do_compile:
        nc.compile()
    return nc


def make_in_maps(preds, gts):
    ones16 = np.ones((2, N), np.float16)
    ident = np.eye(P, dtype=np.float16)
    in_maps = []
    for c in range(N_CORES):
        gb = gts[c * BPC : (c + 1) * BPC]  # x = gts
        pb = preds[c * BPC : (c + 1) * BPC]  # y = preds
        # xq rows: x0(3), x1(3), y0(3), y1(3) — each [3, N] transposed
        xq = np.concatenate(
            [gb.transpose(0, 2, 1), pb.transpose(0, 2, 1)], axis=0
        ).reshape(NT * D, N)
        # wq: per-tensor [NIT, D*P] point-major blocks stacked
        wq = np.concatenate(
            [gb.reshape(BPC, NIT, D * P), pb.reshape(BPC, NIT, D * P)], axis=0
        ).reshape(NT * NIT, D * P)
        in_maps.append(
            {
                "xq": np.ascontiguousarray(xq),
                "wq": np.ascontiguousarray(wq),
                "ones16": ones16,
                "ident": ident,
            }
        )
    return in_maps


_prog = None
last_run_info = {}


def kernel(preds, gts):
    global _prog
    preds = np.ascontiguousarray(np.asarray(preds, dtype=np.float32))
    gts = np.ascontiguousarray(np.asarray(gts, dtype=np.float32))
    assert preds.shape == (B, N, D) and gts.shape == (B, N, D)
    if _prog is None:
        _prog = build_program()
    in_maps = make_in_maps(preds, gts)
    trace = bool(int(os.environ.get("CHAMFER_TRACE", "0")))
    r = run_bass_kernel_spmd(_prog, in_maps, list(range(N_CORES)), trace=trace)
    last_run_info["exec_time_ns"] = r.exec_time_ns
    last_run_info["results"] = r
    total = sum(float(m["out"][0, 0]) for m in r.results)
    return np.asarray(total / float(B * N), dtype=np.float32)
